# revision 23
# baseline (speedup 1.0000x reference)
"""Trainium2 Bass kernel for BackwardMaxPooling2D (F_IBP backward-bounds fold).

Reference computation:
    b_u = maxpool2x2(u_c).reshape(B,1,-1,1); b_l = maxpool2x2(l_c)...
    b_out_u_ = sum_p max(w_u,0)*b_u + sum_p min(w_u,0)*b_l + b_out_u
    b_out_l_ = sum_p max(w_l,0)*b_l + sum_p min(w_l,0)*b_u + b_out_l
    w_zero   = zeros(B,1,n_in,n_out)  (returned twice)

Identity used on device (avoids separate max/min passes):
    max(w,0)*bu + min(w,0)*bl = w*bl + relu(w)*(bu-bl)
so per (batch, tensor) we need two reductions over n_pool:
    v0 . W   and   d . relu(W),  with d = b_u - b_l,
    v0 = b_l for the upper tensor, v0 = b_u for the lower tensor.

Sharding: n_pool = 131072 = (32 pooled rows) x 32 x 128 is split into 8
contiguous chunks of 16384 (4 pooled rows each); core i consumes input rows
[8i, 8i+8) of u_c/l_c and rows [16384i, 16384(i+1)) of w_out_u/w_out_l.
All reductions stay core-local; host adds the 8 partial (32,)-vectors.

On-chip layout (per core, per batch, per tensor): the 16384x32 weight chunk
is loaded contiguously as SBUF (128p, 128, 32) = (spatial q, channel r, j).
The pooled bounds land naturally as (128p spatial, 128 channel).  TensorE
contracts over the 128 spatial partitions; channels are blocked 16 at a
time into lhsT columns with rhs = the matching (128, 16*32=512) W slab,
accumulating a (16, 512) PSUM tile over 8 steps.  Only the block-diagonal
(m == rr) entries are wanted; the host extracts them from the (16, 8, 512)
result tensor (einsum 'mgmj->gj') -- cheaper than any on-device shuffle.
"""

import sys

import numpy as np

sys.path.insert(0, "/opt/trn_rl_repo")

import concourse.tile as tile  # noqa: E402
from concourse import bacc, mybir  # noqa: E402
from concourse.bass_utils import run_bass_kernel_spmd  # noqa: E402

B = 2
H, W_DIM, C = 64, 64, 128
N_OUT = 32
N_CORES = 8
HS = H // N_CORES  # input rows per core (8)
NPOOL_CORE = (HS // 2) * (W_DIM // 2) * C  # 16384 pooled elems per (core, batch)
QP = 128  # spatial positions per (core, batch) = partitions
MR = 16  # channels folded per matmul step (N = MR*32 = 512)
NSTEP = C // MR  # 8 accumulation steps
N_GROUPS = B * 2 * 2  # (batch) x (u/l tensor) x (v0/d vector) = 8

_CACHE = {}


DEFAULT_OPTS = dict(
    pool_mode="h16",  # 'h32': 32 one-row DMAs; 'h16': 16 two-row DMAs
    pool_eng="sync",  # engine issuing pooling DMAs
    w_eng="sync",  # engine issuing W-chunk DMAs
    out_eng="sync",  # engine issuing the result DMA
    w_split=1,  # W chunk DMAs per (b, t)
    bufs_w=4,
    bufs_r=4,
    bufs_ps=8,
    relu_split=True,  # alternate relu between ACT and DVE
    sched="v2",  # v1: sequential; v2: interleaved; v3: fused-W + ring split
    relu_half=True,  # split each relu tile across ACT and DVE halves
    dma_alternate=True,  # alternate DMAs across both HWDGE rings (SP/ACT)
    warmup_mm=0,  # dummy matmuls during the DMA lead-in to pre-warm PE HAM
    ring_mode="alt2",  # 'alt2': alternate SP/ACT; 'pool_gpsimd': pooling on SWDGE
)


def _emit_body(nc, u, l_, w_dram, out, small, wpool, rpool, opool, psum, opts):
    f32 = mybir.dt.float32
    if opts.get("dma_alternate"):
        _engines = [nc.sync, nc.scalar, nc.gpsimd][: opts.get("dma_rings", 2)]
        _ctr = [0]

        def _next_eng():
            e = _engines[_ctr[0] % len(_engines)]
            _ctr[0] += 1
            return e

        pool_eng_f = w_eng_f = out_eng_f = _next_eng
        if opts.get("ring_mode") == "pool_gpsimd":
            pool_eng_f = lambda: nc.gpsimd  # noqa: E731
    else:
        pool_eng = getattr(nc, opts["pool_eng"])
        w_eng = getattr(nc, opts["w_eng"])
        out_eng = getattr(nc, opts["out_eng"])
        pool_eng_f = lambda: pool_eng  # noqa: E731
        w_eng_f = lambda: w_eng  # noqa: E731
        out_eng_f = lambda: out_eng  # noqa: E731
    s_all = opool.tile([16, N_GROUPS, 512], f32)

    # Pre-warm the PE HAM clock gate during the DMA lead-in: dummy matmuls
    # on a scratch tile keep TensorE busy so the real stream runs at 2.4GHz.
    n_warm = opts.get("warmup_mm", 0)
    if n_warm:
        scr = small.tile([QP, 512], f32, tag="warm_scr")
        nc.vector.memset(scr[:], 0.0)
        wps = psum.tile([MR, 512], f32, tag="ps")
        for _ in range(n_warm):
            nc.tensor.matmul(
                wps[:], scr[:, 0:MR], scr[:], start=True, stop=True
            )

    pooled = {}
    dtile = {}

    def emit_pool(b):
        # 2x2 maxpool of u_c / l_c slab b -> (128 spatial, 128 chan)
        for t, src in ((0, u), (1, l_)):
            v4 = small.tile([QP, 2, 2, C], f32, tag=f"v4_{b}{t}")
            for h in range(HS // 2):
                if opts["pool_mode"] == "h16":
                    pool_eng_f().dma_start(
                        out=v4[h * 32 : (h + 1) * 32, :, :, :].rearrange(
                            "w dh dw c -> w dh (dw c)"
                        ),
                        in_=src[b, 2 * h : 2 * h + 2].rearrange(
                            "dh (w dw) c -> w dh (dw c)", dw=2
                        ),
                    )
                else:
                    for dh in range(2):
                        pool_eng_f().dma_start(
                            out=v4[h * 32 : (h + 1) * 32, dh, :, :],
                            in_=src[b, 2 * h + dh].rearrange(
                                "(w dw) c -> w dw c", dw=2
                            ),
                        )
            pb = small.tile([QP, C], f32, tag=f"pooled{b}{t}")
            nc.vector.tensor_reduce(
                out=pb,
                in_=v4[:].rearrange("p dh dw c -> p c (dh dw)"),
                axis=mybir.AxisListType.X,
                op=mybir.AluOpType.max,
            )
            pooled[(b, t)] = pb
        dt_ = small.tile([QP, C], f32, tag=f"d{b}")
        nc.vector.tensor_sub(out=dt_, in0=pooled[(b, 0)], in1=pooled[(b, 1)])
        dtile[b] = dt_

    def emit_w_load(b, t):
        wt = wpool.tile([QP, C, N_OUT], f32, tag="w")
        ws = opts["w_split"]
        qstep = QP // ws
        for k in range(ws):
            w_eng_f().dma_start(
                out=wt[k * qstep : (k + 1) * qstep],
                in_=w_dram[t][b, k * qstep * C : (k + 1) * qstep * C].rearrange(
                    "(q r) j -> q r j", q=qstep
                ),
            )
        return wt

    def emit_relu(b, t, wt):
        rt = rpool.tile([QP, C, N_OUT], f32, tag="r")
        if opts.get("relu_half"):
            half = C // 2
            nc.scalar.activation(
                out=rt[:, :half], in_=wt[:, :half],
                func=mybir.ActivationFunctionType.Relu,
            )
            nc.vector.tensor_scalar_max(
                out=rt[:, half:], in0=wt[:, half:], scalar1=0.0
            )
        elif opts["relu_split"] and (b + t) % 2 == 1:
            nc.vector.tensor_scalar_max(out=rt, in0=wt, scalar1=0.0)
        else:
            nc.scalar.activation(
                out=rt, in_=wt, func=mybir.ActivationFunctionType.Relu
            )
        return rt

    def emit_groups(b, t, wt, rt, g):
        v0 = pooled[(b, 1 - t)]  # t=0 (upper): b_l ; t=1 (lower): b_u
        for lv, rv in ((v0, wt), (dtile[b], rt)):
            ps = psum.tile([MR, 512], f32, tag="ps")
            for s in range(NSTEP):
                r0 = s * MR
                nc.tensor.matmul(
                    ps[:],
                    lv[:, r0 : r0 + MR],
                    rv[:, r0 : r0 + MR, :],
                    start=(s == 0),
                    stop=(s == NSTEP - 1),
                )
            nc.vector.tensor_copy(out=s_all[:, g, :], in_=ps[:])
            g += 1
        return g

    if opts.get("sched") == "v3":
        # Two fused 4MB W DMAs (one per tensor, both batches), explicit ring
        # assignment, PE-dense group order so HAM warms once.
        #   sync ring:   pool(b=0) 8 DMAs, wl, out
        #   scalar ring: wu, pool(b=1) 8 DMAs
        def emit_pool_eng(b, eng):
            for t, src in ((0, u), (1, l_)):
                v4 = small.tile([QP, 2, 2, C], f32, tag=f"v4_{b}{t}")
                for h in range(HS // 2):
                    eng.dma_start(
                        out=v4[h * 32 : (h + 1) * 32, :, :, :].rearrange(
                            "w dh dw c -> w dh (dw c)"
                        ),
                        in_=src[b, 2 * h : 2 * h + 2].rearrange(
                            "dh (w dw) c -> w dh (dw c)", dw=2
                        ),
                    )
                pb = small.tile([QP, C], f32, tag=f"pooled{b}{t}")
                nc.vector.tensor_reduce(
                    out=pb,
                    in_=v4[:].rearrange("p dh dw c -> p c (dh dw)"),
                    axis=mybir.AxisListType.X,
                    op=mybir.AluOpType.max,
                )
                pooled[(b, t)] = pb
            dt_ = small.tile([QP, C], f32, tag=f"d{b}")
            nc.vector.tensor_sub(out=dt_, in0=pooled[(b, 0)], in1=pooled[(b, 1)])
            dtile[b] = dt_

        def emit_w_fused(t, eng):
            wt = wpool.tile([QP, B, C * N_OUT], f32, tag=f"wf{t}")
            eng.dma_start(
                out=wt,
                in_=w_dram[t][:].rearrange("b (q rj) j -> q b (rj j)", q=QP),
            )
            return wt

        def emit_relu_fused(t, wt):
            rt = rpool.tile([QP, B, C * N_OUT], f32, tag=f"rf{t}")
            nc.scalar.activation(
                out=rt[:, 0, :], in_=wt[:, 0, :],
                func=mybir.ActivationFunctionType.Relu,
            )
            nc.vector.tensor_scalar_max(
                out=rt[:, 1, :], in0=wt[:, 1, :], scalar1=0.0
            )
            return rt

        def emit_groups_fused(b, t, wt, rt, g):
            v0 = pooled[(b, 1 - t)]
            for lv, rv in ((v0, wt), (dtile[b], rt)):
                ps = psum.tile([MR, 512], f32, tag="ps")
                for s in range(NSTEP):
                    r0 = s * MR
                    nc.tensor.matmul(
                        ps[:],
                        lv[:, r0 : r0 + MR],
                        rv[:, b, r0 * N_OUT : (r0 + MR) * N_OUT],
                        start=(s == 0),
                        stop=(s == NSTEP - 1),
                    )
                nc.vector.tensor_copy(out=s_all[:, g, :], in_=ps[:])
                g += 1
            return g

        wt_u = emit_w_fused(0, nc.scalar)  # scalar ring first: wu
        emit_pool_eng(0, nc.sync)  # sync ring: batch-0 pooling
        wt_l = emit_w_fused(1, nc.sync)  # sync ring: wl after pooling b0
        emit_pool_eng(1, nc.scalar)  # scalar ring: batch-1 pooling
        wts = {0: wt_u, 1: wt_l}
        rts = {0: emit_relu_fused(0, wt_u), 1: emit_relu_fused(1, wt_l)}
        g = 0
        order = [(0, 0), (0, 1), (1, 0), (1, 1)]  # (b, t): wu b0, wl b0, ...
        gmap = {}
        for b, t in order:
            gmap[(b, t)] = g
            g = emit_groups_fused(b, t, wts[t], rts[t], g)
        opts["_gorder"] = order
    elif opts.get("sched") == "v2":
        # interleave: pool(b) slabs then that batch's W loads; relu+matmuls
        # chase the loads so PE starts early and stays fed.
        g = 0
        wts = {}
        for b in range(B):
            emit_pool(b)
            for t in range(2):
                wts[(b, t)] = emit_w_load(b, t)
            for t in range(2):
                rt = emit_relu(b, t, wts[(b, t)])
                g = emit_groups(b, t, wts[(b, t)], rt, g)
    else:
        for b in range(B):
            emit_pool(b)
        g = 0
        for b in range(B):
            for t in range(2):
                wt = emit_w_load(b, t)
                rt = emit_relu(b, t, wt)
                g = emit_groups(b, t, wt, rt, g)

    out_eng_f().dma_start(out=out[:], in_=s_all[:])


def _build_bass(loop_n=1, **overrides):
    opts = dict(DEFAULT_OPTS)
    opts.update(overrides)
    nc = bacc.Bacc(None, target_bir_lowering=False, debug=False)
    f32 = mybir.dt.float32
    u = nc.dram_tensor("u", [B, HS, W_DIM, C], f32, kind="ExternalInput")
    l_ = nc.dram_tensor("l", [B, HS, W_DIM, C], f32, kind="ExternalInput")
    wu = nc.dram_tensor("wu", [B, NPOOL_CORE, N_OUT], f32, kind="ExternalInput")
    wl = nc.dram_tensor("wl", [B, NPOOL_CORE, N_OUT], f32, kind="ExternalInput")
    out = nc.dram_tensor("out", [16, N_GROUPS, 512], f32, kind="ExternalOutput")
    w_dram = (wu, wl)

    with tile.TileContext(nc) as tc:
        with (
            tc.tile_pool(name="small", bufs=2) as small,
            tc.tile_pool(name="wpool", bufs=opts["bufs_w"]) as wpool,
            tc.tile_pool(name="rpool", bufs=opts["bufs_r"]) as rpool,
            tc.tile_pool(name="opool", bufs=1) as opool,
            tc.tile_pool(name="psum", bufs=opts["bufs_ps"], space="PSUM") as psum,
        ):
            pools = (small, wpool, rpool, opool, psum)
            if loop_n == 1:
                _emit_body(nc, u, l_, w_dram, out, *pools, opts)
            else:
                with tc.For_i(0, loop_n, 1):
                    _emit_body(nc, u, l_, w_dram, out, *pools, opts)
    nc.compile()
    return nc


def _shard_inputs(y, u_c, l_c, w_out_u, w_out_l):
    u_c = np.ascontiguousarray(u_c, dtype=np.float32)
    l_c = np.ascontiguousarray(l_c, dtype=np.float32)
    wu_full = np.ascontiguousarray(w_out_u, dtype=np.float32).reshape(B, -1, N_OUT)
    wl_full = np.ascontiguousarray(w_out_l, dtype=np.float32).reshape(B, -1, N_OUT)
    in_maps = []
    for i in range(N_CORES):
        in_maps.append(
            {
                "u": np.ascontiguousarray(u_c[:, HS * i : HS * (i + 1)]),
                "l": np.ascontiguousarray(l_c[:, HS * i : HS * (i + 1)]),
                "wu": np.ascontiguousarray(
                    wu_full[:, NPOOL_CORE * i : NPOOL_CORE * (i + 1)]
                ),
                "wl": np.ascontiguousarray(
                    wl_full[:, NPOOL_CORE * i : NPOOL_CORE * (i + 1)]
                ),
            }
        )
    return in_maps


def _combine(results, y, b_out_u, b_out_l):
    acc = np.zeros((N_GROUPS, N_OUT), np.float64)
    for i in range(N_CORES):
        r = np.asarray(results[i]["out"])  # (16, N_GROUPS, 512)
        # group g partial_j = sum_m r[m, g, m*32 + j]
        acc += np.einsum("mgmj->gj", r.reshape(MR, N_GROUPS, MR, N_OUT))
    acc = acc.reshape(B, 2, 2, N_OUT)
    res_u = (acc[:, 0, 0] + acc[:, 0, 1]).astype(np.float32)  # b_l.Wu + d.relu(Wu)
    res_l = (acc[:, 1, 0] - acc[:, 1, 1]).astype(np.float32)  # b_u.Wl - d.relu(Wl)

    b_out_u_ = res_u.reshape(B, 1, N_OUT) + b_out_u
    b_out_l_ = res_l.reshape(B, 1, N_OUT) + b_out_l

    n_in = int(np.prod(y.shape[1:]))
    w_zero = np.zeros((B, 1, n_in, N_OUT), np.float32)
    return (w_zero, b_out_u_, w_zero, b_out_l_)


def kernel(y, x_0, u_c, l_c, w_out_u, b_out_u, w_out_l, b_out_l):
    if "nc" not in _CACHE:
        _CACHE["nc"] = _build_bass()
    nc = _CACHE["nc"]
    in_maps = _shard_inputs(y, u_c, l_c, w_out_u, w_out_l)
    res = run_bass_kernel_spmd(nc, in_maps, list(range(N_CORES)))
    return _combine(res.results, y, b_out_u, b_out_l)


# revision 24
# speedup vs baseline: 1.2242x; 1.2242x over previous
"""Trainium2 Bass kernel for BackwardMaxPooling2D (F_IBP backward-bounds fold).

Reference computation:
    b_u = maxpool2x2(u_c).reshape(B,1,-1,1); b_l = maxpool2x2(l_c)...
    b_out_u_ = sum_p max(w_u,0)*b_u + sum_p min(w_u,0)*b_l + b_out_u
    b_out_l_ = sum_p max(w_l,0)*b_l + sum_p min(w_l,0)*b_u + b_out_l
    w_zero   = zeros(B,1,n_in,n_out)  (returned twice)

Identity used on device (avoids separate max/min passes):
    max(w,0)*bu + min(w,0)*bl = w*bl + relu(w)*(bu-bl)
so per (batch, tensor) we need two reductions over n_pool:
    v0 . W   and   d . relu(W),  with d = b_u - b_l,
    v0 = b_l for the upper tensor, v0 = b_u for the lower tensor.

Sharding: n_pool = 131072 = (32 pooled rows) x 32 x 128 is split into 8
contiguous chunks of 16384 (4 pooled rows each); core i consumes input rows
[8i, 8i+8) of u_c/l_c and rows [16384i, 16384(i+1)) of w_out_u/w_out_l.
All reductions stay core-local; host adds the 8 partial (32,)-vectors.

On-chip layout (per core, per batch, per tensor): the 16384x32 weight chunk
is loaded contiguously as SBUF (128p, 128, 32) = (spatial q, channel r, j).
The pooled bounds land naturally as (128p spatial, 128 channel).  TensorE
contracts over the 128 spatial partitions; channels are blocked 16 at a
time into lhsT columns with rhs = the matching (128, 16*32=512) W slab,
accumulating a (16, 512) PSUM tile over 8 steps.  Only the block-diagonal
(m == rr) entries are wanted; the host extracts them from the (16, 8, 512)
result tensor (einsum 'mgmj->gj') -- cheaper than any on-device shuffle.
"""

import sys

import numpy as np

sys.path.insert(0, "/opt/trn_rl_repo")

import concourse.tile as tile  # noqa: E402
from concourse import bacc, mybir  # noqa: E402
from concourse.bass_utils import run_bass_kernel_spmd  # noqa: E402

B = 2
H, W_DIM, C = 64, 64, 128
N_OUT = 32
N_CORES = 8
HS = H // N_CORES  # input rows per core (8)
NPOOL_CORE = (HS // 2) * (W_DIM // 2) * C  # 16384 pooled elems per (core, batch)
QP = 128  # spatial positions per (core, batch) = partitions
MR = 16  # channels folded per matmul step (N = MR*32 = 512)
NSTEP = C // MR  # 8 accumulation steps
N_GROUPS = B * 2 * 2  # (batch) x (u/l tensor) x (v0/d vector) = 8

_CACHE = {}


DEFAULT_OPTS = dict(
    pool_mode="h16",  # 'h32': 32 one-row DMAs; 'h16': 16 two-row DMAs
    pool_eng="sync",  # engine issuing pooling DMAs
    w_eng="sync",  # engine issuing W-chunk DMAs
    out_eng="sync",  # engine issuing the result DMA
    w_split=1,  # W chunk DMAs per (b, t)
    bufs_w=4,
    bufs_r=4,
    bufs_ps=8,
    relu_split=True,  # alternate relu between ACT and DVE
    sched="v2",  # v1: sequential; v2: interleaved; v3: fused-W + ring split
    relu_half=True,  # split each relu tile across ACT and DVE halves
    dma_alternate=True,  # alternate DMAs across both HWDGE rings (SP/ACT)
    warmup_mm=0,  # dummy matmuls during the DMA lead-in to pre-warm PE HAM
    ring_mode="alt2",  # 'alt2': alternate SP/ACT; 'pool_gpsimd': pooling on SWDGE
)


def _emit_body(nc, u, l_, w_dram, out, small, wpool, rpool, opool, psum, opts):
    f32 = mybir.dt.float32
    if opts.get("dma_alternate"):
        _engines = [nc.sync, nc.scalar, nc.gpsimd][: opts.get("dma_rings", 2)]
        _ctr = [0]

        def _next_eng():
            e = _engines[_ctr[0] % len(_engines)]
            _ctr[0] += 1
            return e

        pool_eng_f = w_eng_f = out_eng_f = _next_eng
        if opts.get("ring_mode") == "pool_gpsimd":
            pool_eng_f = lambda: nc.gpsimd  # noqa: E731
    else:
        pool_eng = getattr(nc, opts["pool_eng"])
        w_eng = getattr(nc, opts["w_eng"])
        out_eng = getattr(nc, opts["out_eng"])
        pool_eng_f = lambda: pool_eng  # noqa: E731
        w_eng_f = lambda: w_eng  # noqa: E731
        out_eng_f = lambda: out_eng  # noqa: E731
    s_all = opool.tile([16, N_GROUPS, 512], f32)

    # Pre-warm the PE HAM clock gate during the DMA lead-in: dummy matmuls
    # on a scratch tile keep TensorE busy so the real stream runs at 2.4GHz.
    n_warm = opts.get("warmup_mm", 0)
    if n_warm:
        scr = small.tile([QP, 512], f32, tag="warm_scr")
        nc.vector.memset(scr[:], 0.0)
        wps = psum.tile([MR, 512], f32, tag="ps")
        for _ in range(n_warm):
            nc.tensor.matmul(
                wps[:], scr[:, 0:MR], scr[:], start=True, stop=True
            )

    pooled = {}
    dtile = {}

    def emit_pool(b):
        # 2x2 maxpool of u_c / l_c slab b -> (128 spatial, 128 chan)
        for t, src in ((0, u), (1, l_)):
            v4 = small.tile([QP, 2, 2, C], f32, tag=f"v4_{b}{t}")
            for h in range(HS // 2):
                if opts["pool_mode"] == "h16":
                    pool_eng_f().dma_start(
                        out=v4[h * 32 : (h + 1) * 32, :, :, :].rearrange(
                            "w dh dw c -> w dh (dw c)"
                        ),
                        in_=src[b, 2 * h : 2 * h + 2].rearrange(
                            "dh (w dw) c -> w dh (dw c)", dw=2
                        ),
                    )
                else:
                    for dh in range(2):
                        pool_eng_f().dma_start(
                            out=v4[h * 32 : (h + 1) * 32, dh, :, :],
                            in_=src[b, 2 * h + dh].rearrange(
                                "(w dw) c -> w dw c", dw=2
                            ),
                        )
            pb = small.tile([QP, C], f32, tag=f"pooled{b}{t}")
            nc.vector.tensor_reduce(
                out=pb,
                in_=v4[:].rearrange("p dh dw c -> p c (dh dw)"),
                axis=mybir.AxisListType.X,
                op=mybir.AluOpType.max,
            )
            pooled[(b, t)] = pb
        dt_ = small.tile([QP, C], f32, tag=f"d{b}")
        nc.vector.tensor_sub(out=dt_, in0=pooled[(b, 0)], in1=pooled[(b, 1)])
        dtile[b] = dt_

    def emit_w_load(b, t):
        wt = wpool.tile([QP, C, N_OUT], f32, tag="w")
        ws = opts["w_split"]
        qstep = QP // ws
        for k in range(ws):
            w_eng_f().dma_start(
                out=wt[k * qstep : (k + 1) * qstep],
                in_=w_dram[t][b, k * qstep * C : (k + 1) * qstep * C].rearrange(
                    "(q r) j -> q r j", q=qstep
                ),
            )
        return wt

    def emit_relu(b, t, wt):
        rt = rpool.tile([QP, C, N_OUT], f32, tag="r")
        if opts.get("relu_half"):
            half = C // 2
            nc.scalar.activation(
                out=rt[:, :half], in_=wt[:, :half],
                func=mybir.ActivationFunctionType.Relu,
            )
            nc.vector.tensor_scalar_max(
                out=rt[:, half:], in0=wt[:, half:], scalar1=0.0
            )
        elif opts["relu_split"] and (b + t) % 2 == 1:
            nc.vector.tensor_scalar_max(out=rt, in0=wt, scalar1=0.0)
        else:
            nc.scalar.activation(
                out=rt, in_=wt, func=mybir.ActivationFunctionType.Relu
            )
        return rt

    def emit_groups(b, t, wt, rt, g):
        v0 = pooled[(b, 1 - t)]  # t=0 (upper): b_l ; t=1 (lower): b_u
        for lv, rv in ((v0, wt), (dtile[b], rt)):
            ps = psum.tile([MR, 512], f32, tag="ps")
            for s in range(NSTEP):
                r0 = s * MR
                nc.tensor.matmul(
                    ps[:],
                    lv[:, r0 : r0 + MR],
                    rv[:, r0 : r0 + MR, :],
                    start=(s == 0),
                    stop=(s == NSTEP - 1),
                )
            nc.vector.tensor_copy(out=s_all[:, g, :], in_=ps[:])
            g += 1
        return g

    if opts.get("sched") == "v3":
        # Two fused 4MB W DMAs (one per tensor, both batches), explicit ring
        # assignment, PE-dense group order so HAM warms once.
        #   sync ring:   pool(b=0) 8 DMAs, wl, out
        #   scalar ring: wu, pool(b=1) 8 DMAs
        def emit_pool_eng(b, eng):
            for t, src in ((0, u), (1, l_)):
                v4 = small.tile([QP, 2, 2, C], f32, tag=f"v4_{b}{t}")
                for h in range(HS // 2):
                    eng.dma_start(
                        out=v4[h * 32 : (h + 1) * 32, :, :, :].rearrange(
                            "w dh dw c -> w dh (dw c)"
                        ),
                        in_=src[b, 2 * h : 2 * h + 2].rearrange(
                            "dh (w dw) c -> w dh (dw c)", dw=2
                        ),
                    )
                pb = small.tile([QP, C], f32, tag=f"pooled{b}{t}")
                nc.vector.tensor_reduce(
                    out=pb,
                    in_=v4[:].rearrange("p dh dw c -> p c (dh dw)"),
                    axis=mybir.AxisListType.X,
                    op=mybir.AluOpType.max,
                )
                pooled[(b, t)] = pb
            dt_ = small.tile([QP, C], f32, tag=f"d{b}")
            nc.vector.tensor_sub(out=dt_, in0=pooled[(b, 0)], in1=pooled[(b, 1)])
            dtile[b] = dt_

        def emit_w_fused(t, eng):
            wt = wpool.tile([QP, B, C * N_OUT], f32, tag=f"wf{t}")
            eng.dma_start(
                out=wt,
                in_=w_dram[t][:].rearrange("b (q rj) j -> q b (rj j)", q=QP),
            )
            return wt

        def emit_relu_fused(t, wt):
            rt = rpool.tile([QP, B, C * N_OUT], f32, tag=f"rf{t}")
            nc.scalar.activation(
                out=rt[:, 0, :], in_=wt[:, 0, :],
                func=mybir.ActivationFunctionType.Relu,
            )
            nc.vector.tensor_scalar_max(
                out=rt[:, 1, :], in0=wt[:, 1, :], scalar1=0.0
            )
            return rt

        def emit_groups_fused(b, t, wt, rt, g):
            v0 = pooled[(b, 1 - t)]
            for lv, rv in ((v0, wt), (dtile[b], rt)):
                ps = psum.tile([MR, 512], f32, tag="ps")
                for s in range(NSTEP):
                    r0 = s * MR
                    nc.tensor.matmul(
                        ps[:],
                        lv[:, r0 : r0 + MR],
                        rv[:, b, r0 * N_OUT : (r0 + MR) * N_OUT],
                        start=(s == 0),
                        stop=(s == NSTEP - 1),
                    )
                nc.vector.tensor_copy(out=s_all[:, g, :], in_=ps[:])
                g += 1
            return g

        wt_u = emit_w_fused(0, nc.scalar)  # scalar ring first: wu
        emit_pool_eng(0, nc.sync)  # sync ring: batch-0 pooling
        wt_l = emit_w_fused(1, nc.sync)  # sync ring: wl after pooling b0
        emit_pool_eng(1, nc.scalar)  # scalar ring: batch-1 pooling
        wts = {0: wt_u, 1: wt_l}
        rts = {0: emit_relu_fused(0, wt_u), 1: emit_relu_fused(1, wt_l)}
        g = 0
        order = [(0, 0), (0, 1), (1, 0), (1, 1)]  # (b, t): wu b0, wl b0, ...
        gmap = {}
        for b, t in order:
            gmap[(b, t)] = g
            g = emit_groups_fused(b, t, wts[t], rts[t], g)
        opts["_gorder"] = order
    elif opts.get("sched") == "v2":
        # interleave: pool(b) slabs then that batch's W loads; relu+matmuls
        # chase the loads so PE starts early and stays fed.
        g = 0
        wts = {}
        for b in range(B):
            emit_pool(b)
            for t in range(2):
                wts[(b, t)] = emit_w_load(b, t)
            for t in range(2):
                rt = emit_relu(b, t, wts[(b, t)])
                g = emit_groups(b, t, wts[(b, t)], rt, g)
    else:
        for b in range(B):
            emit_pool(b)
        g = 0
        for b in range(B):
            for t in range(2):
                wt = emit_w_load(b, t)
                rt = emit_relu(b, t, wt)
                g = emit_groups(b, t, wt, rt, g)

    if opts.get("out_split", True):
        nc.sync.dma_start(
            out=out[:, : N_GROUPS // 2, :], in_=s_all[:, : N_GROUPS // 2, :]
        )
        nc.scalar.dma_start(
            out=out[:, N_GROUPS // 2 :, :], in_=s_all[:, N_GROUPS // 2 :, :]
        )
    else:
        out_eng_f().dma_start(out=out[:], in_=s_all[:])


def _build_bass(loop_n=1, **overrides):
    opts = dict(DEFAULT_OPTS)
    opts.update(overrides)
    nc = bacc.Bacc(None, target_bir_lowering=False, debug=False)
    f32 = mybir.dt.float32
    u = nc.dram_tensor("u", [B, HS, W_DIM, C], f32, kind="ExternalInput")
    l_ = nc.dram_tensor("l", [B, HS, W_DIM, C], f32, kind="ExternalInput")
    wu = nc.dram_tensor("wu", [B, NPOOL_CORE, N_OUT], f32, kind="ExternalInput")
    wl = nc.dram_tensor("wl", [B, NPOOL_CORE, N_OUT], f32, kind="ExternalInput")
    out = nc.dram_tensor("out", [16, N_GROUPS, 512], f32, kind="ExternalOutput")
    w_dram = (wu, wl)

    with tile.TileContext(nc) as tc:
        with (
            tc.tile_pool(name="small", bufs=2) as small,
            tc.tile_pool(name="wpool", bufs=opts["bufs_w"]) as wpool,
            tc.tile_pool(name="rpool", bufs=opts["bufs_r"]) as rpool,
            tc.tile_pool(name="opool", bufs=1) as opool,
            tc.tile_pool(name="psum", bufs=opts["bufs_ps"], space="PSUM") as psum,
        ):
            pools = (small, wpool, rpool, opool, psum)
            if loop_n == 1:
                _emit_body(nc, u, l_, w_dram, out, *pools, opts)
            else:
                with tc.For_i(0, loop_n, 1):
                    _emit_body(nc, u, l_, w_dram, out, *pools, opts)
    nc.compile()
    return nc


def _shard_inputs(y, u_c, l_c, w_out_u, w_out_l):
    u_c = np.ascontiguousarray(u_c, dtype=np.float32)
    l_c = np.ascontiguousarray(l_c, dtype=np.float32)
    wu_full = np.ascontiguousarray(w_out_u, dtype=np.float32).reshape(B, -1, N_OUT)
    wl_full = np.ascontiguousarray(w_out_l, dtype=np.float32).reshape(B, -1, N_OUT)
    in_maps = []
    for i in range(N_CORES):
        in_maps.append(
            {
                "u": np.ascontiguousarray(u_c[:, HS * i : HS * (i + 1)]),
                "l": np.ascontiguousarray(l_c[:, HS * i : HS * (i + 1)]),
                "wu": np.ascontiguousarray(
                    wu_full[:, NPOOL_CORE * i : NPOOL_CORE * (i + 1)]
                ),
                "wl": np.ascontiguousarray(
                    wl_full[:, NPOOL_CORE * i : NPOOL_CORE * (i + 1)]
                ),
            }
        )
    return in_maps


def _combine(results, y, b_out_u, b_out_l):
    acc = np.zeros((N_GROUPS, N_OUT), np.float64)
    for i in range(N_CORES):
        r = np.asarray(results[i]["out"])  # (16, N_GROUPS, 512)
        # group g partial_j = sum_m r[m, g, m*32 + j]
        acc += np.einsum("mgmj->gj", r.reshape(MR, N_GROUPS, MR, N_OUT))
    acc = acc.reshape(B, 2, 2, N_OUT)
    res_u = (acc[:, 0, 0] + acc[:, 0, 1]).astype(np.float32)  # b_l.Wu + d.relu(Wu)
    res_l = (acc[:, 1, 0] - acc[:, 1, 1]).astype(np.float32)  # b_u.Wl - d.relu(Wl)

    b_out_u_ = res_u.reshape(B, 1, N_OUT) + b_out_u
    b_out_l_ = res_l.reshape(B, 1, N_OUT) + b_out_l

    n_in = int(np.prod(y.shape[1:]))
    w_zero = np.zeros((B, 1, n_in, N_OUT), np.float32)
    return (w_zero, b_out_u_, w_zero, b_out_l_)


def kernel(y, x_0, u_c, l_c, w_out_u, b_out_u, w_out_l, b_out_l):
    if "nc" not in _CACHE:
        _CACHE["nc"] = _build_bass()
    nc = _CACHE["nc"]
    in_maps = _shard_inputs(y, u_c, l_c, w_out_u, w_out_l)
    res = run_bass_kernel_spmd(nc, in_maps, list(range(N_CORES)))
    return _combine(res.results, y, b_out_u, b_out_l)


# revision 28
# speedup vs baseline: 1.4144x; 1.1554x over previous
"""Trainium2 Bass kernel for BackwardMaxPooling2D (F_IBP backward-bounds fold).

Reference computation:
    b_u = maxpool2x2(u_c).reshape(B,1,-1,1); b_l = maxpool2x2(l_c)...
    b_out_u_ = sum_p max(w_u,0)*b_u + sum_p min(w_u,0)*b_l + b_out_u
    b_out_l_ = sum_p max(w_l,0)*b_l + sum_p min(w_l,0)*b_u + b_out_l
    w_zero   = zeros(B,1,n_in,n_out)  (returned twice)

Identity used on device (avoids separate max/min passes):
    max(w,0)*bu + min(w,0)*bl = w*bl + relu(w)*(bu-bl)
so per (batch, tensor) we need two reductions over n_pool:
    v0 . W   and   d . relu(W),  with d = b_u - b_l,
    v0 = b_l for the upper tensor, v0 = b_u for the lower tensor.

Sharding: n_pool = 131072 = (32 pooled rows) x 32 x 128 is split into 8
contiguous chunks of 16384 (4 pooled rows each); core i consumes input rows
[8i, 8i+8) of u_c/l_c and rows [16384i, 16384(i+1)) of w_out_u/w_out_l.
All reductions stay core-local; host adds the 8 partial (32,)-vectors.

On-chip layout (per core, per batch, per tensor): the 16384x32 weight chunk
is loaded contiguously as SBUF (128p, 128, 32) = (spatial q, channel r, j).
The pooled bounds land naturally as (128p spatial, 128 channel).  TensorE
contracts over the 128 spatial partitions; channels are blocked 16 at a
time into lhsT columns with rhs = the matching (128, 16*32=512) W slab,
accumulating a (16, 512) PSUM tile over 8 steps.  Only the block-diagonal
(m == rr) entries are wanted; the host extracts them from the (16, 8, 512)
result tensor (einsum 'mgmj->gj') -- cheaper than any on-device shuffle.
"""

import sys

import numpy as np

sys.path.insert(0, "/opt/trn_rl_repo")

import concourse.tile as tile  # noqa: E402
from concourse import bacc, mybir  # noqa: E402
from concourse.bass_utils import run_bass_kernel_spmd  # noqa: E402

B = 2
H, W_DIM, C = 64, 64, 128
N_OUT = 32
N_CORES = 8
HS = H // N_CORES  # input rows per core (8)
NPOOL_CORE = (HS // 2) * (W_DIM // 2) * C  # 16384 pooled elems per (core, batch)
QP = 128  # spatial positions per (core, batch) = partitions
MR = 16  # channels folded per matmul step (N = MR*32 = 512)
NSTEP = C // MR  # 8 accumulation steps
N_GROUPS = B * 2 * 2  # (batch) x (u/l tensor) x (v0/d vector) = 8

_CACHE = {}


DEFAULT_OPTS = dict(
    pool_mode="h16",  # 'h32': 32 one-row DMAs; 'h16': 16 two-row DMAs
    pool_eng="sync",  # engine issuing pooling DMAs
    w_eng="sync",  # engine issuing W-chunk DMAs
    out_eng="sync",  # engine issuing the result DMA
    w_split=1,  # W chunk DMAs per (b, t)
    bufs_w=4,
    bufs_r=4,
    bufs_ps=8,
    relu_split=True,  # alternate relu between ACT and DVE
    sched="v2",  # v1: sequential; v2: interleaved; v3: fused-W + ring split
    relu_half=True,  # split each relu tile across ACT and DVE halves
    relu_eng="dve",  # 'dve': all relus on DVE, ACT stays a pure DMA issuer
    dma_alternate=True,  # alternate DMAs across both HWDGE rings (SP/ACT)
    warmup_mm=0,  # dummy matmuls during the DMA lead-in to pre-warm PE HAM
    ring_mode="alt2",  # 'alt2': alternate SP/ACT; 'pool_gpsimd': pooling on SWDGE
)


def _emit_body(nc, u, l_, w_dram, out, small, wpool, rpool, opool, psum, opts):
    f32 = mybir.dt.float32
    if opts.get("dma_alternate"):
        _engines = [nc.sync, nc.scalar, nc.gpsimd][: opts.get("dma_rings", 2)]
        _ctr = [0]

        def _next_eng():
            e = _engines[_ctr[0] % len(_engines)]
            _ctr[0] += 1
            return e

        pool_eng_f = w_eng_f = out_eng_f = _next_eng
        if opts.get("ring_mode") == "pool_gpsimd":
            pool_eng_f = lambda: nc.gpsimd  # noqa: E731
    else:
        pool_eng = getattr(nc, opts["pool_eng"])
        w_eng = getattr(nc, opts["w_eng"])
        out_eng = getattr(nc, opts["out_eng"])
        pool_eng_f = lambda: pool_eng  # noqa: E731
        w_eng_f = lambda: w_eng  # noqa: E731
        out_eng_f = lambda: out_eng  # noqa: E731
    s_all = opool.tile([16, N_GROUPS, 512], f32)

    # Pre-warm the PE HAM clock gate during the DMA lead-in: dummy matmuls
    # on a scratch tile keep TensorE busy so the real stream runs at 2.4GHz.
    n_warm = opts.get("warmup_mm", 0)
    if n_warm:
        scr = small.tile([QP, 512], f32, tag="warm_scr")
        nc.vector.memset(scr[:], 0.0)
        wps = psum.tile([MR, 512], f32, tag="ps")
        for _ in range(n_warm):
            nc.tensor.matmul(
                wps[:], scr[:, 0:MR], scr[:], start=True, stop=True
            )

    pooled = {}
    dtile = {}

    def emit_pool(b):
        # 2x2 maxpool of u_c / l_c slab b -> (128 spatial, 128 chan)
        for t, src in ((0, u), (1, l_)):
            v4 = small.tile([QP, 2, 2, C], f32, tag=f"v4_{b}{t}")
            for h in range(HS // 2):
                if opts["pool_mode"] == "h16":
                    pool_eng_f().dma_start(
                        out=v4[h * 32 : (h + 1) * 32, :, :, :].rearrange(
                            "w dh dw c -> w dh (dw c)"
                        ),
                        in_=src[b, 2 * h : 2 * h + 2].rearrange(
                            "dh (w dw) c -> w dh (dw c)", dw=2
                        ),
                    )
                else:
                    for dh in range(2):
                        pool_eng_f().dma_start(
                            out=v4[h * 32 : (h + 1) * 32, dh, :, :],
                            in_=src[b, 2 * h + dh].rearrange(
                                "(w dw) c -> w dw c", dw=2
                            ),
                        )
            pb = small.tile([QP, C], f32, tag=f"pooled{b}{t}")
            nc.vector.tensor_reduce(
                out=pb,
                in_=v4[:].rearrange("p dh dw c -> p c (dh dw)"),
                axis=mybir.AxisListType.X,
                op=mybir.AluOpType.max,
            )
            pooled[(b, t)] = pb
        dt_ = small.tile([QP, C], f32, tag=f"d{b}")
        nc.vector.tensor_sub(out=dt_, in0=pooled[(b, 0)], in1=pooled[(b, 1)])
        dtile[b] = dt_

    def emit_w_load(b, t):
        wt = wpool.tile([QP, C, N_OUT], f32, tag="w")
        ws = opts["w_split"]
        qstep = QP // ws
        for k in range(ws):
            w_eng_f().dma_start(
                out=wt[k * qstep : (k + 1) * qstep],
                in_=w_dram[t][b, k * qstep * C : (k + 1) * qstep * C].rearrange(
                    "(q r) j -> q r j", q=qstep
                ),
            )
        return wt

    def emit_relu(b, t, wt):
        rt = rpool.tile([QP, C, N_OUT], f32, tag="r")
        if opts.get("relu_eng") == "dve":
            nc.vector.tensor_scalar_max(out=rt, in0=wt, scalar1=0.0)
        elif opts.get("relu_half"):
            half = C // 2
            nc.scalar.activation(
                out=rt[:, :half], in_=wt[:, :half],
                func=mybir.ActivationFunctionType.Relu,
            )
            nc.vector.tensor_scalar_max(
                out=rt[:, half:], in0=wt[:, half:], scalar1=0.0
            )
        elif opts["relu_split"] and (b + t) % 2 == 1:
            nc.vector.tensor_scalar_max(out=rt, in0=wt, scalar1=0.0)
        else:
            nc.scalar.activation(
                out=rt, in_=wt, func=mybir.ActivationFunctionType.Relu
            )
        return rt

    def emit_groups(b, t, wt, rt, g):
        v0 = pooled[(b, 1 - t)]  # t=0 (upper): b_l ; t=1 (lower): b_u
        for lv, rv in ((v0, wt), (dtile[b], rt)):
            ps = psum.tile([MR, 512], f32, tag="ps")
            for s in range(NSTEP):
                r0 = s * MR
                nc.tensor.matmul(
                    ps[:],
                    lv[:, r0 : r0 + MR],
                    rv[:, r0 : r0 + MR, :],
                    start=(s == 0),
                    stop=(s == NSTEP - 1),
                )
            nc.vector.tensor_copy(out=s_all[:, g, :], in_=ps[:])
            g += 1
        return g

    if opts.get("sched") == "v3":
        # Two fused 4MB W DMAs (one per tensor, both batches), explicit ring
        # assignment, PE-dense group order so HAM warms once.
        #   sync ring:   pool(b=0) 8 DMAs, wl, out
        #   scalar ring: wu, pool(b=1) 8 DMAs
        def emit_pool_eng(b, eng):
            for t, src in ((0, u), (1, l_)):
                v4 = small.tile([QP, 2, 2, C], f32, tag=f"v4_{b}{t}")
                for h in range(HS // 2):
                    eng.dma_start(
                        out=v4[h * 32 : (h + 1) * 32, :, :, :].rearrange(
                            "w dh dw c -> w dh (dw c)"
                        ),
                        in_=src[b, 2 * h : 2 * h + 2].rearrange(
                            "dh (w dw) c -> w dh (dw c)", dw=2
                        ),
                    )
                pb = small.tile([QP, C], f32, tag=f"pooled{b}{t}")
                nc.vector.tensor_reduce(
                    out=pb,
                    in_=v4[:].rearrange("p dh dw c -> p c (dh dw)"),
                    axis=mybir.AxisListType.X,
                    op=mybir.AluOpType.max,
                )
                pooled[(b, t)] = pb
            dt_ = small.tile([QP, C], f32, tag=f"d{b}")
            nc.vector.tensor_sub(out=dt_, in0=pooled[(b, 0)], in1=pooled[(b, 1)])
            dtile[b] = dt_

        def emit_w_fused(t, eng):
            wt = wpool.tile([QP, B, C * N_OUT], f32, tag=f"wf{t}")
            eng.dma_start(
                out=wt,
                in_=w_dram[t][:].rearrange("b (q rj) j -> q b (rj j)", q=QP),
            )
            return wt

        def emit_relu_fused(t, wt):
            rt = rpool.tile([QP, B, C * N_OUT], f32, tag=f"rf{t}")
            nc.scalar.activation(
                out=rt[:, 0, :], in_=wt[:, 0, :],
                func=mybir.ActivationFunctionType.Relu,
            )
            nc.vector.tensor_scalar_max(
                out=rt[:, 1, :], in0=wt[:, 1, :], scalar1=0.0
            )
            return rt

        def emit_groups_fused(b, t, wt, rt, g):
            v0 = pooled[(b, 1 - t)]
            for lv, rv in ((v0, wt), (dtile[b], rt)):
                ps = psum.tile([MR, 512], f32, tag="ps")
                for s in range(NSTEP):
                    r0 = s * MR
                    nc.tensor.matmul(
                        ps[:],
                        lv[:, r0 : r0 + MR],
                        rv[:, b, r0 * N_OUT : (r0 + MR) * N_OUT],
                        start=(s == 0),
                        stop=(s == NSTEP - 1),
                    )
                nc.vector.tensor_copy(out=s_all[:, g, :], in_=ps[:])
                g += 1
            return g

        wt_u = emit_w_fused(0, nc.scalar)  # scalar ring first: wu
        emit_pool_eng(0, nc.sync)  # sync ring: batch-0 pooling
        wt_l = emit_w_fused(1, nc.sync)  # sync ring: wl after pooling b0
        emit_pool_eng(1, nc.scalar)  # scalar ring: batch-1 pooling
        wts = {0: wt_u, 1: wt_l}
        rts = {0: emit_relu_fused(0, wt_u), 1: emit_relu_fused(1, wt_l)}
        g = 0
        order = [(0, 0), (0, 1), (1, 0), (1, 1)]  # (b, t): wu b0, wl b0, ...
        gmap = {}
        for b, t in order:
            gmap[(b, t)] = g
            g = emit_groups_fused(b, t, wts[t], rts[t], g)
        opts["_gorder"] = order
    elif opts.get("sched") == "v2" and opts.get("w_fused"):
        # v2 ordering, but one 4MB both-batch DMA per W tensor (2 instead
        # of 4) to cut per-DMA sequencer issue cost.
        emit_pool(0)
        wts = {}
        rts = {}
        for t, eng in ((0, nc.sync), (1, nc.scalar)):
            wt = wpool.tile([QP, B, C * N_OUT], f32, tag=f"wf{t}")
            eng.dma_start(
                out=wt,
                in_=w_dram[t][:].rearrange("b (q rj) j -> q b (rj j)", q=QP),
            )
            wts[t] = wt
        emit_pool(1)
        for t in range(2):
            rt = rpool.tile([QP, B, C * N_OUT], f32, tag=f"rf{t}")
            nc.vector.tensor_scalar_max(out=rt, in0=wts[t], scalar1=0.0)
            rts[t] = rt
        g = 0
        for b in range(B):
            for t in range(2):
                v0 = pooled[(b, 1 - t)]
                for lv, rv in ((v0, wts[t]), (dtile[b], rts[t])):
                    ps = psum.tile([MR, 512], f32, tag="ps")
                    for s in range(NSTEP):
                        r0 = s * MR
                        nc.tensor.matmul(
                            ps[:],
                            lv[:, r0 : r0 + MR],
                            rv[:, b, r0 * N_OUT : (r0 + MR) * N_OUT],
                            start=(s == 0),
                            stop=(s == NSTEP - 1),
                        )
                    nc.vector.tensor_copy(out=s_all[:, g, :], in_=ps[:])
                    g += 1
    elif opts.get("sched") == "v2":
        # interleave: pool(b) slabs then that batch's W loads; relu+matmuls
        # chase the loads so PE starts early and stays fed.
        g = 0
        wts = {}
        for b in range(B):
            emit_pool(b)
            for t in range(2):
                wts[(b, t)] = emit_w_load(b, t)
            for t in range(2):
                rt = emit_relu(b, t, wts[(b, t)])
                g = emit_groups(b, t, wts[(b, t)], rt, g)
    else:
        for b in range(B):
            emit_pool(b)
        g = 0
        for b in range(B):
            for t in range(2):
                wt = emit_w_load(b, t)
                rt = emit_relu(b, t, wt)
                g = emit_groups(b, t, wt, rt, g)

    if opts.get("out_split", False):
        nc.sync.dma_start(
            out=out[:, : N_GROUPS // 2, :], in_=s_all[:, : N_GROUPS // 2, :]
        )
        nc.scalar.dma_start(
            out=out[:, N_GROUPS // 2 :, :], in_=s_all[:, N_GROUPS // 2 :, :]
        )
    else:
        out_eng_f().dma_start(out=out[:], in_=s_all[:])


def _build_bass(loop_n=1, **overrides):
    opts = dict(DEFAULT_OPTS)
    opts.update(overrides)
    nc = bacc.Bacc(None, target_bir_lowering=False, debug=False)
    f32 = mybir.dt.float32
    u = nc.dram_tensor("u", [B, HS, W_DIM, C], f32, kind="ExternalInput")
    l_ = nc.dram_tensor("l", [B, HS, W_DIM, C], f32, kind="ExternalInput")
    wu = nc.dram_tensor("wu", [B, NPOOL_CORE, N_OUT], f32, kind="ExternalInput")
    wl = nc.dram_tensor("wl", [B, NPOOL_CORE, N_OUT], f32, kind="ExternalInput")
    out = nc.dram_tensor("out", [16, N_GROUPS, 512], f32, kind="ExternalOutput")
    w_dram = (wu, wl)

    with tile.TileContext(nc) as tc:
        with (
            tc.tile_pool(name="small", bufs=2) as small,
            tc.tile_pool(name="wpool", bufs=opts["bufs_w"]) as wpool,
            tc.tile_pool(name="rpool", bufs=opts["bufs_r"]) as rpool,
            tc.tile_pool(name="opool", bufs=1) as opool,
            tc.tile_pool(name="psum", bufs=opts["bufs_ps"], space="PSUM") as psum,
        ):
            pools = (small, wpool, rpool, opool, psum)
            if loop_n == 1:
                _emit_body(nc, u, l_, w_dram, out, *pools, opts)
            else:
                with tc.For_i(0, loop_n, 1):
                    _emit_body(nc, u, l_, w_dram, out, *pools, opts)
    nc.compile()
    return nc


def _shard_inputs(y, u_c, l_c, w_out_u, w_out_l):
    u_c = np.ascontiguousarray(u_c, dtype=np.float32)
    l_c = np.ascontiguousarray(l_c, dtype=np.float32)
    wu_full = np.ascontiguousarray(w_out_u, dtype=np.float32).reshape(B, -1, N_OUT)
    wl_full = np.ascontiguousarray(w_out_l, dtype=np.float32).reshape(B, -1, N_OUT)
    in_maps = []
    for i in range(N_CORES):
        in_maps.append(
            {
                "u": np.ascontiguousarray(u_c[:, HS * i : HS * (i + 1)]),
                "l": np.ascontiguousarray(l_c[:, HS * i : HS * (i + 1)]),
                "wu": np.ascontiguousarray(
                    wu_full[:, NPOOL_CORE * i : NPOOL_CORE * (i + 1)]
                ),
                "wl": np.ascontiguousarray(
                    wl_full[:, NPOOL_CORE * i : NPOOL_CORE * (i + 1)]
                ),
            }
        )
    return in_maps


def _combine(results, y, b_out_u, b_out_l):
    acc = np.zeros((N_GROUPS, N_OUT), np.float64)
    for i in range(N_CORES):
        r = np.asarray(results[i]["out"])  # (16, N_GROUPS, 512)
        # group g partial_j = sum_m r[m, g, m*32 + j]
        acc += np.einsum("mgmj->gj", r.reshape(MR, N_GROUPS, MR, N_OUT))
    acc = acc.reshape(B, 2, 2, N_OUT)
    res_u = (acc[:, 0, 0] + acc[:, 0, 1]).astype(np.float32)  # b_l.Wu + d.relu(Wu)
    res_l = (acc[:, 1, 0] - acc[:, 1, 1]).astype(np.float32)  # b_u.Wl - d.relu(Wl)

    b_out_u_ = res_u.reshape(B, 1, N_OUT) + b_out_u
    b_out_l_ = res_l.reshape(B, 1, N_OUT) + b_out_l

    n_in = int(np.prod(y.shape[1:]))
    w_zero = np.zeros((B, 1, n_in, N_OUT), np.float32)
    return (w_zero, b_out_u_, w_zero, b_out_l_)


def kernel(y, x_0, u_c, l_c, w_out_u, b_out_u, w_out_l, b_out_l):
    if "nc" not in _CACHE:
        _CACHE["nc"] = _build_bass()
    nc = _CACHE["nc"]
    in_maps = _shard_inputs(y, u_c, l_c, w_out_u, w_out_l)
    res = run_bass_kernel_spmd(nc, in_maps, list(range(N_CORES)))
    return _combine(res.results, y, b_out_u, b_out_l)
